# revision 45
# baseline (speedup 1.0000x reference)
"""Trainium2 Bass kernel for nn_Activity_Detection: 3-modality bidirectional
LSTM activity head.

Sharding (8 NeuronCores): 4 batch shards (128 rows) x 2 LSTM directions.
Cores 0-3 run the forward LSTMs, cores 4-7 the reverse LSTMs on host
time-reversed features; one SPMD program.

Projection dedup (v3): the fwd core s and rev core s+4 need the *same*
resnet/c3d projections (the rev core's features are the host-time-reversed
copy, so proj_rev[t] == proj_fwd[19-t]). Each core computes only its own
t=0..9 half, the pair AllGathers the halves (replica groups [s, s+4]), and
steps t>=10 read both gathered slots blended with a per-core 0/1 mask input
(fwd wants slot1, rev slot0) — keeping the program SPMD-uniform. This halves
both the projection matmuls and the transposing feature DMAs.

Schedule: phases arranged so the PE never waits on elementwise tails or
on the slow transposing DMAs of the feature loads:
  phase 1: resnet projection half (solo; ww_audio + audio xT behind it),
           then the resnet pjt AllGather
  phase 2: audio recurrence interleaved with the c3d projection half (the
           feed-forward projection matmuls fill audio's elementwise bubbles;
           the c3d weight buffers free early so the resnet gate weights can
           prefetch under the audio tail), then the c3d pjt AllGather
  phase 3: resnet recurrence (c3d gate-weight head chunk prefetches in slack)
  phase 4: c3d recurrence + output head
Gate columns are host-permuted to [i0 f0 g0 o0 | i1 f1 g1 o1] (512-wide
chunks) so each 2048-column half's four PSUM banks finish consecutively and
the elementwise work for half 0 overlaps the matmuls of half 1. Within a
step, the hT-independent x-part matmuls are emitted before the h transposes
of the previous step so the PE always has ready work while the previous
step's elementwise tail drains.
"""

import numpy as np
import ml_dtypes

import concourse.bass as bass
import concourse.bacc as bacc
import concourse.tile as tile
import concourse.mybir as mybir
from concourse.masks import make_identity
from concourse import bass_utils

BF16 = mybir.dt.bfloat16
F32 = mybir.dt.float32
AF = mybir.ActivationFunctionType

B, T = 512, 20
TH = T // 2       # projection half computed locally per core
RES, C3D, AUD, P, H, NCLS = 2048, 4096, 128, 1024, 1024, 200
BS = 128          # batch rows per core
G4 = 4 * H        # 4096 gate dim (columns host-permuted)
NKH = H // 128    # 8 h chunks
PAIR_GROUPS = [[0, 4], [1, 5], [2, 6], [3, 7]]
MODS = ("audio", "resnet", "c3d")
DIMS = {"audio": AUD, "resnet": RES, "c3d": C3D}

TRACE = False            # set by test harness for profiling
LAST_RESULTS = None      # BassKernelResults of the last run (for profiling)


def build_program(has_gate_bias: bool):
    nc = bacc.Bacc("TRN2", target_bir_lowering=False, debug=False, num_devices=8)

    x_d = {m: nc.dram_tensor(f"x_{m}", [T, BS, DIMS[m]], BF16, kind="ExternalInput").ap()
           for m in MODS}
    wt_d = {m: nc.dram_tensor(f"wt_{m}", [DIMS[m], P], BF16, kind="ExternalInput").ap()
            for m in ("resnet", "c3d")}
    bp_d = {m: nc.dram_tensor(f"bp_{m}", [P], F32, kind="ExternalInput").ap()
            for m in ("resnet", "c3d")}
    kd = {"audio": AUD + H, "resnet": P + H, "c3d": P + H}
    ww_d = {m: nc.dram_tensor(f"ww_{m}", [kd[m], G4], BF16, kind="ExternalInput").ap()
            for m in MODS}
    gb_d = {m: nc.dram_tensor(f"gb_{m}", [G4], F32, kind="ExternalInput").ap()
            for m in MODS}
    wout_d = nc.dram_tensor("wout", [H, NCLS], F32, kind="ExternalInput").ap()
    dmask_d = nc.dram_tensor("dmask", [128, 2], F32, kind="ExternalInput").ap()
    out_d = nc.dram_tensor("out_partial", [BS, NCLS], F32, kind="ExternalOutput").ap()
    # own projection half (t < TH) and the pair-gathered both-halves buffer
    pjt_d = {m: nc.dram_tensor(f"pjt_{m}", [TH, NKH, 128, BS], BF16, kind="Internal").ap()
             for m in ("resnet", "c3d")}
    pjg_d = {m: nc.dram_tensor(f"pjg_{m}", [2, TH, NKH, 128, BS], BF16, kind="Internal").ap()
             for m in ("resnet", "c3d")}
    # scratch written late in the audio recurrence; the gpsimd DMA reading it
    # gates the projection AllGathers into the DMA-quiet audio tail
    gate_d = nc.dram_tensor("gate_scratch", [1, 512], BF16, kind="Internal").ap()

    from contextlib import ExitStack
    with tile.TileContext(nc) as tc, ExitStack() as stack:
        const = stack.enter_context(tc.tile_pool(name="const", bufs=1))
        state = stack.enter_context(tc.tile_pool(name="state", bufs=1))
        work = stack.enter_context(tc.tile_pool(name="work", bufs=2))
        tpsum = stack.enter_context(tc.tile_pool(name="tpsum", bufs=2, space="PSUM"))

        ident_bf = const.tile([128, 128], BF16)
        make_identity(nc, ident_bf[:])
        ident_f32 = const.tile([128, 128], F32)
        make_identity(nc, ident_f32[:])
        fused_acc = const.tile([128, H], F32)
        dmask = const.tile([128, 2], F32)
        nc.sync.dma_start(dmask[:], dmask_d[:])

        gb_sb = {}
        if has_gate_bias:
            for m in MODS:
                gb_sb[m] = const.tile([128, G4], F32, tag=f"gb_{m}")
                nc.sync.dma_start(gb_sb[m][:], gb_d[m][None, :].to_broadcast([128, G4]))

        # ---------- shared recurrence-step emission ----------
        def ew_half(m, t, j, G, c_st, h_bf):
            """Elementwise for gate-column half j given its 4 PSUM banks
            G = [i, f, g, o]."""
            sl = slice(j * 512, (j + 1) * 512)

            def gin(b):
                src = G[b][:]
                if has_gate_bias:
                    gs = work.tile([128, 512], F32, tag="gs")
                    nc.vector.tensor_add(
                        gs[:], src, gb_sb[m][:, (j * 4 + b) * 512:(j * 4 + b + 1) * 512])
                    src = gs[:]
                return src

            sf = work.tile([128, 512], F32, tag="sf")
            nc.scalar.activation(sf[:], gin(1), AF.Sigmoid)
            if t > 0:
                nc.vector.tensor_mul(c_st[:, sl], sf[:], c_st[:, sl])
            si = work.tile([128, 512], F32, tag="si")
            nc.scalar.activation(si[:], gin(0), AF.Sigmoid)
            tg = work.tile([128, 512], F32, tag="tg")
            nc.scalar.activation(tg[:], gin(2), AF.Tanh)
            if t > 0:
                tmp = work.tile([128, 512], F32, tag="sf")
                nc.vector.tensor_mul(tmp[:], si[:], tg[:])
                nc.vector.tensor_add(c_st[:, sl], c_st[:, sl], tmp[:])
            else:
                nc.vector.tensor_mul(c_st[:, sl], si[:], tg[:])
            tc_t = work.tile([128, 512], F32, tag="tg")
            nc.scalar.activation(tc_t[:], c_st[:, sl], AF.Tanh)
            so = work.tile([128, 512], F32, tag="sf")
            nc.scalar.activation(so[:], gin(3), AF.Sigmoid)
            if t < T - 1:
                nc.vector.tensor_mul(h_bf[:, sl], so[:], tc_t[:])
                if m == "audio" and t == 15 and j == 1:
                    # late-gate for the collectives (see gate_d)
                    nc.gpsimd.dma_start(gate_d[:], h_bf[0:1, sl])
            else:
                if m == "audio":
                    nc.vector.tensor_mul(fused_acc[:, sl], so[:], tc_t[:])
                else:
                    hf = work.tile([128, 512], F32, tag="si")
                    nc.vector.tensor_mul(hf[:], so[:], tc_t[:])
                    nc.vector.tensor_mul(fused_acc[:, sl], fused_acc[:, sl], hf[:])

        def emit_recurrence(m, gp, ww_at, n_kx, st_x_fn, filler=None):
            """One full 20-step recurrence for modality m.

            gp: PSUM tile pool for the gate banks.
            ww_at(k): AP of the [128, G4] weight row-chunk k (k < n_kx: x-part,
                      k >= n_kx: h-part).
            st_x_fn(t, k): stationary [128,128] x chunk for step t.
            filler(t): optional callback emitting independent PE work.
            """
            n_k = n_kx + NKH
            hT = state.tile([128, NKH, 128], BF16, tag="hT")
            c_st = state.tile([128, H], F32, tag="c_st")
            h_bf = state.tile([128, H], BF16, tag="h_bf")

            for t in range(T):
                if filler is not None:
                    filler(t)
                G0 = [gp.tile([128, 512], F32, tag="g", name=f"g_{m}_{t}_0_{b}")
                      for b in range(4)]
                # half 0, x-part (independent of hT(t-1)), k-outer b-inner
                for k in range(n_kx):
                    for b in range(4):
                        nc.tensor.matmul(
                            G0[b][:], st_x_fn(t, k), ww_at(k)[:, b * 512:(b + 1) * 512],
                            start=(k == 0), stop=(t == 0 and k == n_kx - 1))
                if t > 0:
                    # h transposes of the previous step (wait on ew(t-1))
                    for k in range(NKH):
                        tp = tpsum.tile([128, 512], F32, tag="tp",
                                        name=f"tp_{m}_{t}_{k}")
                        tpv = tp[:, 0:128].bitcast(BF16)[:, 0:128]
                        nc.tensor.transpose(
                            tpv, h_bf[:, k * 128:(k + 1) * 128], ident_bf[:])
                        nc.vector.tensor_copy(hT[:, k - 0, :], tpv)
                    # half 0, h-part
                    for k in range(n_kx, n_k):
                        for b in range(4):
                            nc.tensor.matmul(
                                G0[b][:], hT[:, k - n_kx, :],
                                ww_at(k)[:, b * 512:(b + 1) * 512],
                                start=False, stop=(k == n_k - 1))
                ew_half(m, t, 0, G0, c_st, h_bf)
                # half 1: bank-outer k-inner (staggers first-writes past the
                # half-0 elementwise reads of the rotating PSUM banks)
                G1 = [gp.tile([128, 512], F32, tag="g", name=f"g_{m}_{t}_1_{b}")
                      for b in range(4)]
                last = (n_kx if t == 0 else n_k) - 1
                for b in range(4):
                    for k in range(last + 1):
                        nc.tensor.matmul(
                            G1[b][:],
                            (st_x_fn(t, k) if k < n_kx else hT[:, k - n_kx, :]),
                            ww_at(k)[:, (4 + b) * 512:(4 + b + 1) * 512],
                            start=(k == 0), stop=(k == last))
                ew_half(m, t, 1, G1, c_st, h_bf)

        # pjt streaming for resnet/c3d recurrences (eager prefetch of t=0).
        # t < TH: own local half. t >= TH: blend of the two gathered slots
        # (slot1 for fwd cores, slot0 for rev cores, selected by dmask).
        def make_pjt_stream(m, pjs):
            tiles = {}

            def prefetch(t):
                if t >= T or t in tiles:
                    return
                if t < TH:
                    pt = pjs.tile([128, NKH, BS], BF16, tag="pjt")
                    nc.sync.dma_start(
                        pt[:], pjt_d[m][t].rearrange("mo p b -> p mo b"))
                    tiles[t] = pt
                else:
                    pa = pjs.tile([128, NKH, BS], BF16, tag="pjA", bufs=2)
                    pb = pjs.tile([128, NKH, BS], BF16, tag="pjB", bufs=2)
                    px = pjs.tile([128, NKH, BS], BF16, tag="pjX", bufs=2)
                    nc.sync.dma_start(
                        pa[:], pjg_d[m][0, T - 1 - t].rearrange("mo p b -> p mo b"))
                    nc.sync.dma_start(
                        pb[:], pjg_d[m][1, T - 1 - t].rearrange("mo p b -> p mo b"))
                    # px = pb*m + pa*(1-m)
                    nc.vector.tensor_scalar_mul(px[:], pb[:], dmask[:, 0:1])
                    nc.vector.scalar_tensor_tensor(
                        px[:], pa[:], dmask[:, 1:2], px[:],
                        mybir.AluOpType.mult, mybir.AluOpType.add)
                    tiles[t] = px

            prefetch(0)

            def st_x(t, k):
                prefetch(t)
                if k == 0:
                    prefetch(t + 1)
                for tt in [tt for tt in tiles if tt < t - 1]:
                    del tiles[tt]
                return tiles[t][:, k, :]

            return st_x

        # ---------------- phase 1: resnet projection (own half) ----------------
        with tc.tile_pool(name="wwp_a", bufs=1) as wwp_a:
            xt_a = wwp_a.tile([128, T, 128], BF16, tag="xt_a")
            with nc.named_scope("p1_resproj"):
                with (
                    tc.tile_pool(name="wtp_r", bufs=1) as wtp_r,
                    tc.tile_pool(name="xtp_r", bufs=3) as xtp_r,
                    tc.tile_pool(name="evp_r", bufs=3) as evp_r,
                    tc.tile_pool(name="projp_r", bufs=3, space="PSUM") as projp_r,
                ):
                    bp_r = const.tile([128, NKH], F32, tag="bp_resnet")
                    nc.sync.dma_start(
                        bp_r[:], bp_d["resnet"].rearrange("(mo p) -> p mo", p=128))
                    dk_r = RES // 128
                    wt_r = wtp_r.tile([128, dk_r, P], BF16, tag="wt_r")
                    wtr = wt_d["resnet"].rearrange("(ko p) n -> p ko n", p=128)
                    for k in range(dk_r):
                        nc.sync.dma_start(wt_r[:, k], wtr[:, k])
                    wwa = wwp_a.tile([128, 1 + NKH, G4], BF16, tag="ww_a")
                    wwar = ww_d["audio"].rearrange("(ko p) n -> p ko n", p=128)
                    RCH = 256
                    n_rch = TH * BS // RCH
                    xt_tiles_r = {}

                    def fetch_xt_r(r):
                        if r < n_rch and r not in xt_tiles_r:
                            xt = xtp_r.tile([128, dk_r, RCH], BF16, tag="xt")
                            for tt in range(RCH // BS):
                                t = (r * RCH) // BS + tt
                                nc.sync.dma_start_transpose(
                                    xt[:, :, tt * BS:(tt + 1) * BS],
                                    x_d["resnet"][t])
                            xt_tiles_r[r] = xt

                    fetch_xt_r(0)
                    fetch_xt_r(1)
                    for r in range(n_rch):
                        fetch_xt_r(r + 2)
                        xt = xt_tiles_r.pop(r)
                        for mo in range(NKH):
                            pp = projp_r.tile([128, 512], F32, tag="pp",
                                              name=f"pjr_{r}_{mo}")
                            for k in range(dk_r):
                                nc.tensor.matmul(
                                    pp[:, 0:RCH], wt_r[:, k, mo * 128:(mo + 1) * 128],
                                    xt[:, k, :], start=(k == 0), stop=(k == dk_r - 1))
                            ev = evp_r.tile([128, RCH], BF16, tag="ev")
                            nc.scalar.activation(ev[:], pp[:, 0:RCH], AF.Identity,
                                                 bias=bp_r[:, mo:mo + 1])
                            for tt in range(RCH // BS):
                                t = (r * RCH) // BS + tt
                                nc.sync.dma_start(
                                    pjt_d["resnet"][t, mo],
                                    ev[:, tt * BS:(tt + 1) * BS])
                        # phase-2 loads ride the phase-1 tail, spread so they
                        # never starve the xt feed
                        if r in (0, 1, 2):
                            for k in range(3 * r, 3 * r + 3):
                                nc.sync.dma_start(wwa[:, k], wwar[:, k])
                        if r == 3:
                            for t in range(T):
                                nc.sync.dma_start_transpose(
                                    xt_a[:, t, :], x_d["audio"][t])


            # -------- phase 2: audio recurrence || c3d projection --------
            with nc.named_scope("p2_audio_c3dproj"):
                with (
                    tc.tile_pool(name="wtp_c", bufs=1) as wtp_c,
                    tc.tile_pool(name="xtp_c", bufs=2) as xtp_c,
                    tc.tile_pool(name="evp_c", bufs=2) as evp_c,
                    tc.tile_pool(name="projp_c", bufs=2, space="PSUM") as projp_c,
                    tc.tile_pool(name="gp2", bufs=4, space="PSUM") as gp2,
                ):
                    bp_c = const.tile([128, NKH], F32, tag="bp_c3d")
                    nc.sync.dma_start(
                        bp_c[:], bp_d["c3d"].rearrange("(mo p) -> p mo", p=128))
                    dk_c = C3D // 128
                    wt_c = wtp_c.tile([128, dk_c, P], BF16, tag="wt_c")
                    wtc = wt_d["c3d"].rearrange("(ko p) n -> p ko n", p=128)
                    for k in range(dk_c):
                        nc.sync.dma_start(wt_c[:, k], wtc[:, k])

                    RC2 = 256  # c3d proj rows per slab (2 time steps)
                    n_half = 2 * (TH * BS // RC2)  # 10 half-slabs (4 mo each)
                    xt_tiles_c = {}

                    def fetch_xt_c(r):
                        if r < n_half // 2 and r not in xt_tiles_c:
                            xt = xtp_c.tile([128, dk_c, RC2], BF16, tag="xt")
                            for tt in range(RC2 // BS):
                                t = (r * RC2) // BS + tt
                                nc.sync.dma_start_transpose(
                                    xt[:, :, tt * BS:(tt + 1) * BS], x_d["c3d"][t])
                            xt_tiles_c[r] = xt

                    def c3d_half_slab(h):
                        r, half = h // 2, h % 2
                        fetch_xt_c(r)
                        if half == 1:
                            fetch_xt_c(r + 1)
                        xt = xt_tiles_c[r]
                        for mo in range(half * 4, half * 4 + 4):
                            pp = projp_c.tile([128, 512], F32, tag="pp",
                                              name=f"pjc_{r}_{mo}")
                            for k in range(dk_c):
                                nc.tensor.matmul(
                                    pp[:, 0:RC2], wt_c[:, k, mo * 128:(mo + 1) * 128],
                                    xt[:, k, :], start=(k == 0), stop=(k == dk_c - 1))
                            ev = evp_c.tile([128, RC2], BF16, tag="ev")
                            nc.scalar.activation(ev[:], pp[:, 0:RC2], AF.Identity,
                                                 bias=bp_c[:, mo:mo + 1])
                            for tt in range(RC2 // BS):
                                t = (r * RC2) // BS + tt
                                nc.sync.dma_start(
                                    pjt_d["c3d"][t, mo],
                                    ev[:, tt * BS:(tt + 1) * BS])

                    # one half-slab per step from t=2 (wt_c streams in during
                    # t=0..1); done by t=11 so the c3d buffers free early for
                    # the resnet gate-weight prefetch
                    hctr = [0]

                    def filler(t):
                        if t == 1:
                            fetch_xt_c(0)   # warm the first slab's feed
                        if 2 <= t and hctr[0] < n_half:
                            c3d_half_slab(hctr[0])
                            hctr[0] += 1

                    emit_recurrence(
                        "audio", gp2,
                        ww_at=lambda k: wwa[:, k],
                        n_kx=1,
                        st_x_fn=lambda t, k: xt_a[:, t, :],
                        filler=filler)
                    while hctr[0] < n_half:
                        c3d_half_slab(hctr[0])
                        hctr[0] += 1
                # pair-exchange the projection halves.  gpsimd runs its
                # instructions in order, and the gate_d DMA above waits on
                # audio's t=15 state, so both collectives execute in the
                # audio tail / early resnet recurrence — after the wt/xt
                # feature loads and the ww_r prefetch, and long before their
                # first consumers (res t>=10 at ~900us, c3d at ~1500us).
                nc.gpsimd.collective_compute(
                    "AllGather", mybir.AluOpType.bypass,
                    replica_groups=PAIR_GROUPS,
                    ins=[pjt_d["resnet"].opt()],
                    outs=[pjg_d["resnet"].opt()])
                nc.gpsimd.collective_compute(
                    "AllGather", mybir.AluOpType.bypass,
                    replica_groups=PAIR_GROUPS,
                    ins=[pjt_d["c3d"].opt()],
                    outs=[pjg_d["c3d"].opt()])

        # wwp_a closed: audio gate weights freed.
        with (
            tc.tile_pool(name="wwcx", bufs=1) as wwcx,
            tc.tile_pool(name="pjs", bufs=3) as pjs,
        ):
            # ---------------- phase 3: resnet recurrence ----------------
            with nc.named_scope("p3_resrec"):
                with (
                    tc.tile_pool(name="wwp_r", bufs=1) as wwp_r,
                    tc.tile_pool(name="gp3", bufs=6, space="PSUM") as gp3,
                ):
                    st_res = make_pjt_stream("resnet", pjs)
                    # x-part chunks are stored at the HIGH tile offsets: the
                    # low offsets of this 128KB range overlap the audio gate
                    # weights (freed only when the audio recurrence ends),
                    # while the high offsets overlap the c3d projection
                    # buffers which free ~150us earlier — so the x-part loads
                    # start early and t=0 of the resnet recurrence is fed.
                    n_k_r = P // 128 + NKH
                    ww_r = wwp_r.tile([128, n_k_r, G4], BF16, tag="ww_r")
                    wwrr = ww_d["resnet"].rearrange("(ko p) n -> p ko n", p=128)

                    def ww_r_at(k):
                        return ww_r[:, (k + NKH) % n_k_r]

                    for k in list(range(NKH)) + list(range(NKH, n_k_r)):
                        nc.sync.dma_start(ww_r_at(k), wwrr[:, k])
                    # prefetch first 4 c3d gate-weight chunks into phase-3 slack
                    NCX = 4
                    ww_c_x4 = wwcx.tile([128, NCX, G4], BF16, tag="ww_c_x4")
                    wwcr = ww_d["c3d"].rearrange("(ko p) n -> p ko n", p=128)
                    for k in range(NCX):
                        nc.sync.dma_start(ww_c_x4[:, k], wwcr[:, k])

                    emit_recurrence(
                        "resnet", gp3,
                        ww_at=ww_r_at,
                        n_kx=P // 128,
                        st_x_fn=st_res)

            # ---------------- phase 4: c3d recurrence + head ----------------
            with nc.named_scope("p4_c3drec"):
                with (
                    tc.tile_pool(name="wwp_c", bufs=1) as wwp_c,
                    tc.tile_pool(name="gp4", bufs=6, space="PSUM") as gp4,
                ):
                    st_c3d = make_pjt_stream("c3d", pjs)
                    n_rest = P // 128 + NKH - NCX
                    ww_c_rest = wwp_c.tile([128, n_rest, G4], BF16, tag="ww_c_rest")
                    for k in range(n_rest):
                        nc.sync.dma_start(ww_c_rest[:, k], wwcr[:, k + NCX])

                    with tc.tile_pool(name="fin", bufs=1) as fin:
                        wo = fin.tile([128, NKH, NCLS], F32)
                        nc.sync.dma_start(
                            wo[:], wout_d.rearrange("(ko p) n -> p ko n", p=128))

                        emit_recurrence(
                            "c3d", gp4,
                            ww_at=lambda k: (ww_c_x4[:, k] if k < NCX
                                             else ww_c_rest[:, k - NCX]),
                            n_kx=P // 128,
                            st_x_fn=st_c3d)

                        # ---------------- output head ----------------
                        ops = gp4.tile([128, 512], F32, tag="g", name="out_ps")
                        for k in range(NKH):
                            tp = tpsum.tile([128, 512], F32, tag="tp",
                                            name=f"ft_{k}")
                            nc.tensor.transpose(
                                tp[:, 0:128], fused_acc[:, k * 128:(k + 1) * 128],
                                ident_f32[:])
                            ft = fin.tile([128, 128], F32, tag="ft", bufs=2)
                            nc.vector.tensor_copy(ft[:], tp[:, 0:128])
                            nc.tensor.matmul(ops[:, 0:NCLS], ft[:], wo[:, k, :],
                                             start=(k == 0), stop=(k == NKH - 1))
                        osb = fin.tile([128, NCLS], F32, tag="osb")
                        nc.vector.tensor_copy(osb[:], ops[:, 0:NCLS])
                        nc.sync.dma_start(out_d[:], osb[:])

    nc.compile()
    return nc


def _bf16(a):
    return np.ascontiguousarray(a).astype(ml_dtypes.bfloat16)


# gate-column permutation: [i0 f0 g0 o0 i1 f1 g1 o1] (512-wide chunks)
_GPERM = np.concatenate(
    [np.arange(b * H + j * 512, b * H + j * 512 + 512)
     for j in (0, 1) for b in range(4)])


def host_prep(inputs):
    f = np.float32
    xs = {"audio": inputs["audio_features"], "resnet": inputs["resnet_features"],
          "c3d": inputs["c3d_features"]}
    xt = {m: np.swapaxes(np.asarray(v, f), 0, 1) for m, v in xs.items()}

    wt = {"resnet": _bf16(np.asarray(inputs["W_resnet"], f).T),
          "c3d": _bf16(np.asarray(inputs["W_c3d"], f).T)}
    bp = {"resnet": np.asarray(inputs["b_resnet"], f),
          "c3d": np.asarray(inputs["b_c3d"], f)}

    dirs = {}
    has_gate_bias = False
    for d in ("fwd", "rev"):
        ww = {}
        gb = {}
        for m in MODS:
            wih = np.asarray(inputs[f"{m}_{d}_Wih"], f)
            whh = np.asarray(inputs[f"{m}_{d}_Whh"], f)
            bih = np.asarray(inputs[f"{m}_{d}_bih"], f)
            bhh = np.asarray(inputs[f"{m}_{d}_bhh"], f)
            if m == "audio":
                wa = np.asarray(inputs["W_audio"], f)
                wcomb = wih @ wa                        # (4H, AUD)
                wwm = np.concatenate([wcomb.T, whh.T], axis=0)
                gbm = (wih @ np.asarray(inputs["b_audio"], f) + bih + bhh).astype(f)
            else:
                wwm = np.concatenate([wih.T, whh.T], axis=0)
                gbm = (bih + bhh).astype(f)
            ww[m] = _bf16(wwm[:, _GPERM])
            gb[m] = np.ascontiguousarray(gbm[_GPERM])
            if np.any(gb[m] != 0):
                has_gate_bias = True
        wout_half = (np.asarray(inputs["W_out"], f)[:, :H].T if d == "fwd"
                     else np.asarray(inputs["W_out"], f)[:, H:].T)
        dirs[d] = {"ww": ww, "gb": gb, "wout": np.ascontiguousarray(wout_half)}

    in_maps = []
    for core in range(8):
        d = "fwd" if core < 4 else "rev"
        s = core % 4
        rows = slice(s * BS, (s + 1) * BS)
        im = {}
        for m in MODS:
            xm = xt[m][:, rows]
            if d == "rev":
                xm = xm[::-1]
            im[f"x_{m}"] = _bf16(xm)
            im[f"ww_{m}"] = dirs[d]["ww"][m]
            im[f"gb_{m}"] = dirs[d]["gb"][m]
        for m in ("resnet", "c3d"):
            im[f"wt_{m}"] = wt[m]
            im[f"bp_{m}"] = bp[m]
        im["wout"] = dirs[d]["wout"]
        # blend mask: fwd cores read gathered slot1 (the rev half), rev
        # cores slot0.  col0 = m, col1 = 1-m.
        mval = 1.0 if d == "fwd" else 0.0
        im["dmask"] = np.tile(np.array([[mval, 1.0 - mval]], np.float32), (128, 1))
        in_maps.append(im)
    return in_maps, has_gate_bias


def assemble(results, inputs):
    out = np.zeros((B, NCLS), np.float32)
    for s in range(4):
        rows = slice(s * BS, (s + 1) * BS)
        out[rows] = results[s]["out_partial"] + results[4 + s]["out_partial"]
    out += np.asarray(inputs["b_out"], np.float32)[None, :]
    return out


def kernel(**inputs):
    global LAST_RESULTS
    in_maps, has_gate_bias = host_prep(inputs)
    nc = build_program(has_gate_bias)
    res = bass_utils.run_bass_kernel_spmd(
        nc, in_maps, core_ids=list(range(8)), trace=TRACE)
    LAST_RESULTS = res
    return assemble(res.results, inputs)


# revision 47
# speedup vs baseline: 1.0104x; 1.0104x over previous
"""Trainium2 Bass kernel for nn_Activity_Detection: 3-modality bidirectional
LSTM activity head.

Sharding (8 NeuronCores): 4 batch shards (128 rows) x 2 LSTM directions.
Cores 0-3 run the forward LSTMs, cores 4-7 the reverse LSTMs on host
time-reversed features; one SPMD program.

Projection dedup (v3): the fwd core s and rev core s+4 need the *same*
resnet/c3d projections (the rev core's features are the host-time-reversed
copy, so proj_rev[t] == proj_fwd[19-t]). Each core computes only its own
t=0..9 half, the pair AllGathers the halves (replica groups [s, s+4]), and
steps t>=10 read both gathered slots blended with a per-core 0/1 mask input
(fwd wants slot1, rev slot0) — keeping the program SPMD-uniform. This halves
both the projection matmuls and the transposing feature DMAs.

Schedule: phases arranged so the PE never waits on elementwise tails or
on the slow transposing DMAs of the feature loads:
  phase 1: resnet projection half (solo; ww_audio + audio xT behind it),
           then the resnet pjt AllGather
  phase 2: audio recurrence interleaved with the c3d projection half (the
           feed-forward projection matmuls fill audio's elementwise bubbles;
           the c3d weight buffers free early so the resnet gate weights can
           prefetch under the audio tail), then the c3d pjt AllGather
  phase 3: resnet recurrence (c3d gate-weight head chunk prefetches in slack)
  phase 4: c3d recurrence + output head
Gate columns are host-permuted to [i0 f0 g0 o0 | i1 f1 g1 o1] (512-wide
chunks) so each 2048-column half's four PSUM banks finish consecutively and
the elementwise work for half 0 overlaps the matmuls of half 1. Within a
step, the hT-independent x-part matmuls are emitted before the h transposes
of the previous step so the PE always has ready work while the previous
step's elementwise tail drains.
"""

import numpy as np
import ml_dtypes

import concourse.bass as bass
import concourse.bacc as bacc
import concourse.tile as tile
import concourse.mybir as mybir
from concourse.masks import make_identity
from concourse import bass_utils

BF16 = mybir.dt.bfloat16
F32 = mybir.dt.float32
AF = mybir.ActivationFunctionType

B, T = 512, 20
TH = T // 2       # projection half computed locally per core
RES, C3D, AUD, P, H, NCLS = 2048, 4096, 128, 1024, 1024, 200
BS = 128          # batch rows per core
G4 = 4 * H        # 4096 gate dim (columns host-permuted)
NKH = H // 128    # 8 h chunks
PAIR_GROUPS = [[0, 4], [1, 5], [2, 6], [3, 7]]
MODS = ("audio", "resnet", "c3d")
DIMS = {"audio": AUD, "resnet": RES, "c3d": C3D}

TRACE = False            # set by test harness for profiling
LAST_RESULTS = None      # BassKernelResults of the last run (for profiling)


def build_program(has_gate_bias: bool):
    nc = bacc.Bacc("TRN2", target_bir_lowering=False, debug=False, num_devices=8)

    x_d = {m: nc.dram_tensor(f"x_{m}", [T, BS, DIMS[m]], BF16, kind="ExternalInput").ap()
           for m in MODS}
    wt_d = {m: nc.dram_tensor(f"wt_{m}", [DIMS[m], P], BF16, kind="ExternalInput").ap()
            for m in ("resnet", "c3d")}
    bp_d = {m: nc.dram_tensor(f"bp_{m}", [P], F32, kind="ExternalInput").ap()
            for m in ("resnet", "c3d")}
    kd = {"audio": AUD + H, "resnet": P + H, "c3d": P + H}
    ww_d = {m: nc.dram_tensor(f"ww_{m}", [kd[m], G4], BF16, kind="ExternalInput").ap()
            for m in MODS}
    gb_d = {m: nc.dram_tensor(f"gb_{m}", [G4], F32, kind="ExternalInput").ap()
            for m in MODS}
    wout_d = nc.dram_tensor("wout", [H, NCLS], F32, kind="ExternalInput").ap()
    dmask_d = nc.dram_tensor("dmask", [128, 2], F32, kind="ExternalInput").ap()
    out_d = nc.dram_tensor("out_partial", [BS, NCLS], F32, kind="ExternalOutput").ap()
    # own projection half (t < TH) and the pair-gathered both-halves buffer
    pjt_d = {m: nc.dram_tensor(f"pjt_{m}", [TH, NKH, 128, BS], BF16, kind="Internal").ap()
             for m in ("resnet", "c3d")}
    pjg_d = {m: nc.dram_tensor(f"pjg_{m}", [2, TH, NKH, 128, BS], BF16, kind="Internal").ap()
             for m in ("resnet", "c3d")}
    # scratch written late in the audio recurrence; the gpsimd DMA reading it
    # gates the projection AllGathers into the DMA-quiet audio tail
    gate_d = nc.dram_tensor("gate_scratch", [1, 512], BF16, kind="Internal").ap()

    from contextlib import ExitStack
    with tile.TileContext(nc) as tc, ExitStack() as stack:
        const = stack.enter_context(tc.tile_pool(name="const", bufs=1))
        state = stack.enter_context(tc.tile_pool(name="state", bufs=1))
        work = stack.enter_context(tc.tile_pool(name="work", bufs=2))
        tpsum = stack.enter_context(tc.tile_pool(name="tpsum", bufs=2, space="PSUM"))

        ident_bf = const.tile([128, 128], BF16)
        make_identity(nc, ident_bf[:])
        ident_f32 = const.tile([128, 128], F32)
        make_identity(nc, ident_f32[:])
        fused_acc = const.tile([128, H], F32)
        dmask = const.tile([128, 2], F32)
        nc.sync.dma_start(dmask[:], dmask_d[:])

        gb_sb = {}
        if has_gate_bias:
            for m in MODS:
                gb_sb[m] = const.tile([128, G4], F32, tag=f"gb_{m}")
                nc.sync.dma_start(gb_sb[m][:], gb_d[m][None, :].to_broadcast([128, G4]))

        # ---------- shared recurrence-step emission ----------
        def ew_half(m, t, j, G, c_st, h_bf):
            """Elementwise for gate-column half j given its 4 PSUM banks
            G = [i, f, g, o]."""
            sl = slice(j * 512, (j + 1) * 512)

            def gin(b):
                src = G[b][:]
                if has_gate_bias:
                    gs = work.tile([128, 512], F32, tag="gs")
                    nc.vector.tensor_add(
                        gs[:], src, gb_sb[m][:, (j * 4 + b) * 512:(j * 4 + b + 1) * 512])
                    src = gs[:]
                return src

            sf = work.tile([128, 512], F32, tag="sf")
            nc.scalar.activation(sf[:], gin(1), AF.Sigmoid)
            if t > 0:
                nc.vector.tensor_mul(c_st[:, sl], sf[:], c_st[:, sl])
            si = work.tile([128, 512], F32, tag="si")
            nc.scalar.activation(si[:], gin(0), AF.Sigmoid)
            tg = work.tile([128, 512], F32, tag="tg")
            nc.scalar.activation(tg[:], gin(2), AF.Tanh)
            if t > 0:
                tmp = work.tile([128, 512], F32, tag="sf")
                nc.vector.tensor_mul(tmp[:], si[:], tg[:])
                nc.vector.tensor_add(c_st[:, sl], c_st[:, sl], tmp[:])
            else:
                nc.vector.tensor_mul(c_st[:, sl], si[:], tg[:])
            tc_t = work.tile([128, 512], F32, tag="tg")
            nc.scalar.activation(tc_t[:], c_st[:, sl], AF.Tanh)
            so = work.tile([128, 512], F32, tag="sf")
            nc.scalar.activation(so[:], gin(3), AF.Sigmoid)
            if t < T - 1:
                nc.vector.tensor_mul(h_bf[:, sl], so[:], tc_t[:])
                if m == "resnet" and t == 2 and j == 1:
                    # late-gate for the collectives (see gate_d): fires early
                    # in the resnet recurrence, after the ww_r load window,
                    # ~200us before the first gathered-slot consumer (t=10)
                    nc.gpsimd.dma_start(gate_d[:], h_bf[0:1, sl])
            else:
                if m == "audio":
                    nc.vector.tensor_mul(fused_acc[:, sl], so[:], tc_t[:])
                else:
                    hf = work.tile([128, 512], F32, tag="si")
                    nc.vector.tensor_mul(hf[:], so[:], tc_t[:])
                    nc.vector.tensor_mul(fused_acc[:, sl], fused_acc[:, sl], hf[:])

        def emit_recurrence(m, gp, ww_at, n_kx, st_x_fn, filler=None):
            """One full 20-step recurrence for modality m.

            gp: PSUM tile pool for the gate banks.
            ww_at(k): AP of the [128, G4] weight row-chunk k (k < n_kx: x-part,
                      k >= n_kx: h-part).
            st_x_fn(t, k): stationary [128,128] x chunk for step t.
            filler(t): optional callback emitting independent PE work.
            """
            n_k = n_kx + NKH
            hT = state.tile([128, NKH, 128], BF16, tag="hT")
            c_st = state.tile([128, H], F32, tag="c_st")
            h_bf = state.tile([128, H], BF16, tag="h_bf")

            for t in range(T):
                if filler is not None:
                    filler(t)
                G0 = [gp.tile([128, 512], F32, tag="g", name=f"g_{m}_{t}_0_{b}")
                      for b in range(4)]
                # half 0, x-part (independent of hT(t-1)), k-outer b-inner
                for k in range(n_kx):
                    for b in range(4):
                        nc.tensor.matmul(
                            G0[b][:], st_x_fn(t, k), ww_at(k)[:, b * 512:(b + 1) * 512],
                            start=(k == 0), stop=(t == 0 and k == n_kx - 1))
                if t > 0:
                    # h transposes of the previous step (wait on ew(t-1))
                    for k in range(NKH):
                        tp = tpsum.tile([128, 512], F32, tag="tp",
                                        name=f"tp_{m}_{t}_{k}")
                        tpv = tp[:, 0:128].bitcast(BF16)[:, 0:128]
                        nc.tensor.transpose(
                            tpv, h_bf[:, k * 128:(k + 1) * 128], ident_bf[:])
                        nc.vector.tensor_copy(hT[:, k - 0, :], tpv)
                    # half 0, h-part
                    for k in range(n_kx, n_k):
                        for b in range(4):
                            nc.tensor.matmul(
                                G0[b][:], hT[:, k - n_kx, :],
                                ww_at(k)[:, b * 512:(b + 1) * 512],
                                start=False, stop=(k == n_k - 1))
                ew_half(m, t, 0, G0, c_st, h_bf)
                # half 1: bank-outer k-inner (staggers first-writes past the
                # half-0 elementwise reads of the rotating PSUM banks)
                G1 = [gp.tile([128, 512], F32, tag="g", name=f"g_{m}_{t}_1_{b}")
                      for b in range(4)]
                last = (n_kx if t == 0 else n_k) - 1
                for b in range(4):
                    for k in range(last + 1):
                        nc.tensor.matmul(
                            G1[b][:],
                            (st_x_fn(t, k) if k < n_kx else hT[:, k - n_kx, :]),
                            ww_at(k)[:, (4 + b) * 512:(4 + b + 1) * 512],
                            start=(k == 0), stop=(k == last))
                ew_half(m, t, 1, G1, c_st, h_bf)

        # pjt streaming for resnet/c3d recurrences (eager prefetch of t=0).
        # t < TH: own local half. t >= TH: blend of the two gathered slots
        # (slot1 for fwd cores, slot0 for rev cores, selected by dmask).
        def make_pjt_stream(m, pjs):
            tiles = {}

            def prefetch(t):
                if t >= T or t in tiles:
                    return
                if t < TH:
                    pt = pjs.tile([128, NKH, BS], BF16, tag="pjt")
                    nc.sync.dma_start(
                        pt[:], pjt_d[m][t].rearrange("mo p b -> p mo b"))
                    tiles[t] = pt
                else:
                    pa = pjs.tile([128, NKH, BS], BF16, tag="pjA", bufs=2)
                    pb = pjs.tile([128, NKH, BS], BF16, tag="pjB", bufs=2)
                    px = pjs.tile([128, NKH, BS], BF16, tag="pjX", bufs=2)
                    nc.sync.dma_start(
                        pa[:], pjg_d[m][0, T - 1 - t].rearrange("mo p b -> p mo b"))
                    nc.sync.dma_start(
                        pb[:], pjg_d[m][1, T - 1 - t].rearrange("mo p b -> p mo b"))
                    # px = pb*m + pa*(1-m)
                    nc.vector.tensor_scalar_mul(px[:], pb[:], dmask[:, 0:1])
                    nc.vector.scalar_tensor_tensor(
                        px[:], pa[:], dmask[:, 1:2], px[:],
                        mybir.AluOpType.mult, mybir.AluOpType.add)
                    tiles[t] = px

            prefetch(0)

            def st_x(t, k):
                prefetch(t)
                if k == 0:
                    prefetch(t + 1)
                for tt in [tt for tt in tiles if tt < t - 1]:
                    del tiles[tt]
                return tiles[t][:, k, :]

            return st_x

        # ---------------- phase 1: resnet projection (own half) ----------------
        with tc.tile_pool(name="wwp_a", bufs=1) as wwp_a:
            xt_a = wwp_a.tile([128, T, 128], BF16, tag="xt_a")
            with nc.named_scope("p1_resproj"):
                with (
                    tc.tile_pool(name="wtp_r", bufs=1) as wtp_r,
                    tc.tile_pool(name="xtp_r", bufs=3) as xtp_r,
                    tc.tile_pool(name="evp_r", bufs=3) as evp_r,
                    tc.tile_pool(name="projp_r", bufs=3, space="PSUM") as projp_r,
                ):
                    bp_r = const.tile([128, NKH], F32, tag="bp_resnet")
                    nc.sync.dma_start(
                        bp_r[:], bp_d["resnet"].rearrange("(mo p) -> p mo", p=128))
                    dk_r = RES // 128
                    wt_r = wtp_r.tile([128, dk_r, P], BF16, tag="wt_r")
                    wtr = wt_d["resnet"].rearrange("(ko p) n -> p ko n", p=128)
                    for k in range(dk_r):
                        nc.sync.dma_start(wt_r[:, k], wtr[:, k])
                    wwa = wwp_a.tile([128, 1 + NKH, G4], BF16, tag="ww_a")
                    wwar = ww_d["audio"].rearrange("(ko p) n -> p ko n", p=128)
                    RCH = 256
                    for r in range(TH * BS // RCH):
                        xt = xtp_r.tile([128, dk_r, RCH], BF16, tag="xt")
                        for tt in range(RCH // BS):
                            t = (r * RCH) // BS + tt
                            nc.sync.dma_start_transpose(
                                xt[:, :, tt * BS:(tt + 1) * BS], x_d["resnet"][t])
                        for mo in range(NKH):
                            pp = projp_r.tile([128, 512], F32, tag="pp",
                                              name=f"pjr_{r}_{mo}")
                            for k in range(dk_r):
                                nc.tensor.matmul(
                                    pp[:, 0:RCH], wt_r[:, k, mo * 128:(mo + 1) * 128],
                                    xt[:, k, :], start=(k == 0), stop=(k == dk_r - 1))
                            ev = evp_r.tile([128, RCH], BF16, tag="ev")
                            nc.scalar.activation(ev[:], pp[:, 0:RCH], AF.Identity,
                                                 bias=bp_r[:, mo:mo + 1])
                            for tt in range(RCH // BS):
                                t = (r * RCH) // BS + tt
                                nc.sync.dma_start(
                                    pjt_d["resnet"][t, mo],
                                    ev[:, tt * BS:(tt + 1) * BS])
                        # phase-2 loads ride the phase-1 tail, spread so they
                        # never starve the xt feed
                        if r in (1, 2, 3):
                            for k in range(3 * (r - 1), 3 * r):
                                nc.sync.dma_start(wwa[:, k], wwar[:, k])
                        if r == 4:
                            for t in range(T):
                                nc.sync.dma_start_transpose(
                                    xt_a[:, t, :], x_d["audio"][t])


            # -------- phase 2: audio recurrence || c3d projection --------
            with nc.named_scope("p2_audio_c3dproj"):
                with (
                    tc.tile_pool(name="wtp_c", bufs=1) as wtp_c,
                    tc.tile_pool(name="xtp_c", bufs=2) as xtp_c,
                    tc.tile_pool(name="evp_c", bufs=2) as evp_c,
                    tc.tile_pool(name="projp_c", bufs=2, space="PSUM") as projp_c,
                    tc.tile_pool(name="gp2", bufs=4, space="PSUM") as gp2,
                ):
                    bp_c = const.tile([128, NKH], F32, tag="bp_c3d")
                    nc.sync.dma_start(
                        bp_c[:], bp_d["c3d"].rearrange("(mo p) -> p mo", p=128))
                    dk_c = C3D // 128
                    wt_c = wtp_c.tile([128, dk_c, P], BF16, tag="wt_c")
                    wtc = wt_d["c3d"].rearrange("(ko p) n -> p ko n", p=128)
                    for k in range(dk_c):
                        nc.sync.dma_start(wt_c[:, k], wtc[:, k])

                    RC2 = 256  # c3d proj rows per slab (2 time steps)
                    n_half = 2 * (TH * BS // RC2)  # 10 half-slabs (4 mo each)
                    cur_xt = {}

                    def c3d_half_slab(h):
                        r, half = h // 2, h % 2
                        if half == 0:
                            xt = xtp_c.tile([128, dk_c, RC2], BF16, tag="xt")
                            for tt in range(RC2 // BS):
                                t = (r * RC2) // BS + tt
                                nc.sync.dma_start_transpose(
                                    xt[:, :, tt * BS:(tt + 1) * BS], x_d["c3d"][t])
                            cur_xt["xt"] = xt
                        xt = cur_xt["xt"]
                        for mo in range(half * 4, half * 4 + 4):
                            pp = projp_c.tile([128, 512], F32, tag="pp",
                                              name=f"pjc_{r}_{mo}")
                            for k in range(dk_c):
                                nc.tensor.matmul(
                                    pp[:, 0:RC2], wt_c[:, k, mo * 128:(mo + 1) * 128],
                                    xt[:, k, :], start=(k == 0), stop=(k == dk_c - 1))
                            ev = evp_c.tile([128, RC2], BF16, tag="ev")
                            nc.scalar.activation(ev[:], pp[:, 0:RC2], AF.Identity,
                                                 bias=bp_c[:, mo:mo + 1])
                            for tt in range(RC2 // BS):
                                t = (r * RC2) // BS + tt
                                nc.sync.dma_start(
                                    pjt_d["c3d"][t, mo],
                                    ev[:, tt * BS:(tt + 1) * BS])

                    # one half-slab per step from t=2 (wt_c streams in during
                    # t=0..1); done by t=11 so the c3d buffers free early for
                    # the resnet gate-weight prefetch
                    hctr = [0]

                    def filler(t):
                        if 2 <= t and hctr[0] < n_half:
                            c3d_half_slab(hctr[0])
                            hctr[0] += 1

                    emit_recurrence(
                        "audio", gp2,
                        ww_at=lambda k: wwa[:, k],
                        n_kx=1,
                        st_x_fn=lambda t, k: xt_a[:, t, :],
                        filler=filler)
                    while hctr[0] < n_half:
                        c3d_half_slab(hctr[0])
                        hctr[0] += 1
                # pair-exchange the projection halves.  gpsimd runs its
                # instructions in order, and the gate_d DMA above waits on
                # audio's t=15 state, so both collectives execute in the
                # audio tail / early resnet recurrence — after the wt/xt
                # feature loads and the ww_r prefetch, and long before their
                # first consumers (res t>=10 at ~900us, c3d at ~1500us).
                nc.gpsimd.collective_compute(
                    "AllGather", mybir.AluOpType.bypass,
                    replica_groups=PAIR_GROUPS,
                    ins=[pjt_d["resnet"].opt()],
                    outs=[pjg_d["resnet"].opt()])
                nc.gpsimd.collective_compute(
                    "AllGather", mybir.AluOpType.bypass,
                    replica_groups=PAIR_GROUPS,
                    ins=[pjt_d["c3d"].opt()],
                    outs=[pjg_d["c3d"].opt()])

        # wwp_a closed: audio gate weights freed.
        with (
            tc.tile_pool(name="wwcx", bufs=1) as wwcx,
            tc.tile_pool(name="pjs", bufs=3) as pjs,
        ):
            # ---------------- phase 3: resnet recurrence ----------------
            with nc.named_scope("p3_resrec"):
                with (
                    tc.tile_pool(name="wwp_r", bufs=1) as wwp_r,
                    tc.tile_pool(name="gp3", bufs=6, space="PSUM") as gp3,
                ):
                    st_res = make_pjt_stream("resnet", pjs)
                    ww_r = wwp_r.tile([128, P // 128 + NKH, G4], BF16, tag="ww_r")
                    wwrr = ww_d["resnet"].rearrange("(ko p) n -> p ko n", p=128)
                    for k in range(P // 128 + NKH):
                        nc.sync.dma_start(ww_r[:, k], wwrr[:, k])
                    # prefetch first 4 c3d gate-weight chunks into phase-3 slack
                    NCX = 4
                    ww_c_x4 = wwcx.tile([128, NCX, G4], BF16, tag="ww_c_x4")
                    wwcr = ww_d["c3d"].rearrange("(ko p) n -> p ko n", p=128)
                    for k in range(NCX):
                        nc.sync.dma_start(ww_c_x4[:, k], wwcr[:, k])

                    emit_recurrence(
                        "resnet", gp3,
                        ww_at=lambda k: ww_r[:, k],
                        n_kx=P // 128,
                        st_x_fn=st_res)

            # ---------------- phase 4: c3d recurrence + head ----------------
            with nc.named_scope("p4_c3drec"):
                with (
                    tc.tile_pool(name="wwp_c", bufs=1) as wwp_c,
                    tc.tile_pool(name="gp4", bufs=6, space="PSUM") as gp4,
                ):
                    st_c3d = make_pjt_stream("c3d", pjs)
                    n_rest = P // 128 + NKH - NCX
                    ww_c_rest = wwp_c.tile([128, n_rest, G4], BF16, tag="ww_c_rest")
                    for k in range(n_rest):
                        nc.sync.dma_start(ww_c_rest[:, k], wwcr[:, k + NCX])

                    with tc.tile_pool(name="fin", bufs=1) as fin:
                        wo = fin.tile([128, NKH, NCLS], F32)
                        nc.sync.dma_start(
                            wo[:], wout_d.rearrange("(ko p) n -> p ko n", p=128))

                        emit_recurrence(
                            "c3d", gp4,
                            ww_at=lambda k: (ww_c_x4[:, k] if k < NCX
                                             else ww_c_rest[:, k - NCX]),
                            n_kx=P // 128,
                            st_x_fn=st_c3d)

                        # ---------------- output head ----------------
                        ops = gp4.tile([128, 512], F32, tag="g", name="out_ps")
                        for k in range(NKH):
                            tp = tpsum.tile([128, 512], F32, tag="tp",
                                            name=f"ft_{k}")
                            nc.tensor.transpose(
                                tp[:, 0:128], fused_acc[:, k * 128:(k + 1) * 128],
                                ident_f32[:])
                            ft = fin.tile([128, 128], F32, tag="ft", bufs=2)
                            nc.vector.tensor_copy(ft[:], tp[:, 0:128])
                            nc.tensor.matmul(ops[:, 0:NCLS], ft[:], wo[:, k, :],
                                             start=(k == 0), stop=(k == NKH - 1))
                        osb = fin.tile([128, NCLS], F32, tag="osb")
                        nc.vector.tensor_copy(osb[:], ops[:, 0:NCLS])
                        nc.sync.dma_start(out_d[:], osb[:])

    nc.compile()
    return nc


def _bf16(a):
    return np.ascontiguousarray(a).astype(ml_dtypes.bfloat16)


# gate-column permutation: [i0 f0 g0 o0 i1 f1 g1 o1] (512-wide chunks)
_GPERM = np.concatenate(
    [np.arange(b * H + j * 512, b * H + j * 512 + 512)
     for j in (0, 1) for b in range(4)])


def host_prep(inputs):
    f = np.float32
    xs = {"audio": inputs["audio_features"], "resnet": inputs["resnet_features"],
          "c3d": inputs["c3d_features"]}
    xt = {m: np.swapaxes(np.asarray(v, f), 0, 1) for m, v in xs.items()}

    wt = {"resnet": _bf16(np.asarray(inputs["W_resnet"], f).T),
          "c3d": _bf16(np.asarray(inputs["W_c3d"], f).T)}
    bp = {"resnet": np.asarray(inputs["b_resnet"], f),
          "c3d": np.asarray(inputs["b_c3d"], f)}

    dirs = {}
    has_gate_bias = False
    for d in ("fwd", "rev"):
        ww = {}
        gb = {}
        for m in MODS:
            wih = np.asarray(inputs[f"{m}_{d}_Wih"], f)
            whh = np.asarray(inputs[f"{m}_{d}_Whh"], f)
            bih = np.asarray(inputs[f"{m}_{d}_bih"], f)
            bhh = np.asarray(inputs[f"{m}_{d}_bhh"], f)
            if m == "audio":
                wa = np.asarray(inputs["W_audio"], f)
                wcomb = wih @ wa                        # (4H, AUD)
                wwm = np.concatenate([wcomb.T, whh.T], axis=0)
                gbm = (wih @ np.asarray(inputs["b_audio"], f) + bih + bhh).astype(f)
            else:
                wwm = np.concatenate([wih.T, whh.T], axis=0)
                gbm = (bih + bhh).astype(f)
            ww[m] = _bf16(wwm[:, _GPERM])
            gb[m] = np.ascontiguousarray(gbm[_GPERM])
            if np.any(gb[m] != 0):
                has_gate_bias = True
        wout_half = (np.asarray(inputs["W_out"], f)[:, :H].T if d == "fwd"
                     else np.asarray(inputs["W_out"], f)[:, H:].T)
        dirs[d] = {"ww": ww, "gb": gb, "wout": np.ascontiguousarray(wout_half)}

    in_maps = []
    for core in range(8):
        d = "fwd" if core < 4 else "rev"
        s = core % 4
        rows = slice(s * BS, (s + 1) * BS)
        im = {}
        for m in MODS:
            xm = xt[m][:, rows]
            if d == "rev":
                xm = xm[::-1]
            im[f"x_{m}"] = _bf16(xm)
            im[f"ww_{m}"] = dirs[d]["ww"][m]
            im[f"gb_{m}"] = dirs[d]["gb"][m]
        for m in ("resnet", "c3d"):
            im[f"wt_{m}"] = wt[m]
            im[f"bp_{m}"] = bp[m]
        im["wout"] = dirs[d]["wout"]
        # blend mask: fwd cores read gathered slot1 (the rev half), rev
        # cores slot0.  col0 = m, col1 = 1-m.
        mval = 1.0 if d == "fwd" else 0.0
        im["dmask"] = np.tile(np.array([[mval, 1.0 - mval]], np.float32), (128, 1))
        in_maps.append(im)
    return in_maps, has_gate_bias


def assemble(results, inputs):
    out = np.zeros((B, NCLS), np.float32)
    for s in range(4):
        rows = slice(s * BS, (s + 1) * BS)
        out[rows] = results[s]["out_partial"] + results[4 + s]["out_partial"]
    out += np.asarray(inputs["b_out"], np.float32)[None, :]
    return out


def kernel(**inputs):
    global LAST_RESULTS
    in_maps, has_gate_bias = host_prep(inputs)
    nc = build_program(has_gate_bias)
    res = bass_utils.run_bass_kernel_spmd(
        nc, in_maps, core_ids=list(range(8)), trace=TRACE)
    LAST_RESULTS = res
    return assemble(res.results, inputs)
